# revision 10
# baseline (speedup 1.0000x reference)
"""Trainium2 Bass kernel for nn_DocumentLevelAttention (B=16, L=1024, D=1024, H=4, DK=DV=256).

Strategy: data-parallel over batch across 8 NeuronCores (2 batch elems/core).
Per-core dataflow (all matmuls fp32r, full PE rate):

  Inputs are pre-transposed on host: qT/kT/vT [e, d, l] so every matmul
  operand has its contraction dim on partitions with NO on-device transposes:
    q_sT [k, q] = w_q.T-chunks @ qT   (lhsT = w chunks, natural)
    k_sT [k, s]  likewise; v_s [s, dv] (lhsT = vT chunks, rhs = w_v, natural)
    scoresT [s, q] = k_sT-chunks.T @ q_sT          (both natural)
    exp_T = ACT Exp(scores/32)  [s, q]
    sums[q] = ones.T @ exp_T  (PE row-sum); recip via DVE approx (2 ULP)
    recip broadcast to all partitions via GPSIMD partition_broadcast
    attns_T = exp_T * recip_bc  (DVE TT, free-dim q matches recip_bc)
    out_uT [dv, q] = v_s-chunks.T @ attns_T        (natural, normalized)
    final [q, d] = concat_uT-chunks.T @ proj_w     (natural)
    LN over free dim d with bn_stats; sigma=exp(0.5*ln(var*N/(N-1))) (one ACT
    table set: natural_log_exp covers Exp+Log, no table thrash)

  attns is produced transposed [s, q]; the host fixes the layout at gather
  time (pure data movement, part of unsharding).
"""
import sys

sys.path.insert(0, "/opt/trn_rl_repo")

import numpy as np

B, L, D = 16, 1024, 1024
H, DK, DV = 4, 256, 256
LN_EPS = 1e-3
NCORES = 8
BLOC = B // NCORES  # 2 batch elems per core
P = 128

_cache = {}


def _build(has_bias: bool, has_ln: bool):
    import concourse.bacc as bacc
    import concourse.mybir as mybir
    import concourse.tile as tile
    import concourse.tile_utils as tile_utils

    # leave headroom above the stale 192K cap (224K phys / 208K usable)
    if getattr(tile_utils, "max_sbuf_usage", 0) < 200 * 1024:
        try:
            tile_utils.max_sbuf_usage = 200 * 1024
        except Exception:
            pass

    # Force Exp and Ln into ONE ACT table set (natural_log_exp_and_others):
    # without this the table chooser alternates exp_and_others <-> ln sets,
    # paying ~2.7us per switch 30+ times.
    if False and not getattr(bacc, "_act_tables_filtered", False):
        _orig_tables = bacc.get_activation_tables

        def _filtered_tables(arch):
            t = _orig_tables(arch)
            return {k: v for k, v in t.items()
                    if k not in ("exp_and_others", "exp_and_friends")}

        bacc.get_activation_tables = _filtered_tables
        bacc._act_tables_filtered = True

    f32 = mybir.dt.float32
    f32r = mybir.dt.float32r
    AF = mybir.ActivationFunctionType
    ALU = mybir.AluOpType

    nc = bacc.Bacc(None)

    qT = nc.declare_dram_parameter("qT", [BLOC, D, L], f32r, isOutput=False)
    kT = nc.declare_dram_parameter("kT", [BLOC, D, L], f32r, isOutput=False)
    vT = nc.declare_dram_parameter("vT", [BLOC, D, L], f32r, isOutput=False)
    qres = nc.declare_dram_parameter("qres", [BLOC, L, D], f32, isOutput=False)
    wq = nc.declare_dram_parameter("wq", [D, H * DK], f32r, isOutput=False)
    wk = nc.declare_dram_parameter("wk", [D, H * DK], f32r, isOutput=False)
    wv = nc.declare_dram_parameter("wv", [D, H * DV], f32r, isOutput=False)
    pw = nc.declare_dram_parameter("pw", [H * DV, D], f32r, isOutput=False)
    ones_d = nc.declare_dram_parameter("ones", [P, 1], f32r, isOutput=False)
    if has_bias:
        pb_d = nc.declare_dram_parameter("pb", [1, D], f32, isOutput=False)
    if has_ln:
        la_d = nc.declare_dram_parameter("la", [1, D], f32, isOutput=False)
        lb_d = nc.declare_dram_parameter("lb", [1, D], f32, isOutput=False)
    out_d = nc.declare_dram_parameter("out", [BLOC, L, D], f32, isOutput=True)
    # attns stored transposed per (elem, head): [s, q]; host untransposes.
    at_d = nc.declare_dram_parameter("attns_t", [BLOC, H, L, L], f32, isOutput=True)

    NB = 512  # matmul free-dim block
    DC = D // P  # 8 contraction chunks
    SP = L // P  # 8 s-partition tiles
    QT4 = 4  # q-tiles per half

    with tile.TileContext(nc) as tc:
        with (
            tc.tile_pool(name="consts", bufs=1) as consts,
            tc.tile_pool(name="bigres", bufs=1) as bigres,
            tc.tile_pool(name="big16", bufs=4) as big16,
            tc.tile_pool(name="w512", bufs=4) as w512,
            tc.tile_pool(name="w1024", bufs=2) as w1024,
            tc.tile_pool(name="rows", bufs=3) as rows,
            tc.tile_pool(name="rbcp", bufs=2) as rbcp,
            tc.tile_pool(name="lnp", bufs=2) as lnp,
            tc.tile_pool(name="qrp", bufs=2) as qrp,
            tc.tile_pool(name="statp", bufs=4) as statp,
            tc.tile_pool(name="pp", bufs=8, space="PSUM") as pp,
        ):
            ones_t = consts.tile([P, 1], f32r, name="ones_t")
            nc.sync.dma_start(ones_t[:], ones_d[:])
            if has_bias:
                pb_bc = consts.tile([P, D], f32, name="pb_bc")
                nc.gpsimd.partition_broadcast(pb_bc[:], pb_d[:])
            if has_ln:
                la_bc = consts.tile([P, D], f32, name="la_bc")
                lb_bc = consts.tile([P, D], f32, name="lb_bc")
                nc.gpsimd.partition_broadcast(la_bc[:], la_d[:])
                nc.gpsimd.partition_broadcast(lb_bc[:], lb_d[:])

            for e in range(BLOC):
                # ---------------- Phase A: projections ----------------
                q_sT = bigres.tile([P, DC, L], f32r, name="q_sT", tag="q_sT")
                k_sT = bigres.tile([P, DC, L], f32r, name="k_sT", tag="k_sT")
                v_s = bigres.tile([P, SP, H * DV], f32r, name="v_s", tag="v_s")

                for src, wsrc, dst in ((qT, wq, q_sT), (kT, wk, k_sT)):
                    src3 = src[e].rearrange("(dc p) l -> p dc l", p=P)
                    xr = []
                    for half in range(2):
                        x = big16.tile([P, DC, NB], f32r, name=f"xr{half}", tag="big16")
                        for d in range(DC):
                            nc.sync.dma_start(
                                x[:, d, :],
                                src3[:, d, half * NB:(half + 1) * NB])
                        xr.append(x)
                    for kpg in range(2):
                        pm = [pp.tile([P, NB], f32, name=f"pmA{i}", tag="pp")
                              for i in range(8)]
                        for d in range(DC):
                            wc = w512.tile([P, NB], f32r, name="wcA", tag="w512")
                            nc.sync.dma_start(
                                wc[:], wsrc[d * P:(d + 1) * P, kpg * NB:(kpg + 1) * NB])
                            for half in range(2):
                                for i in range(4):
                                    nc.tensor.matmul(
                                        pm[half * 4 + i][:],
                                        wc[:, i * P:(i + 1) * P],
                                        xr[half][:, d, :],
                                        start=(d == 0), stop=(d == DC - 1))
                        for half in range(2):
                            for i in range(4):
                                nc.any.tensor_copy(
                                    out=dst[:, kpg * 4 + i, half * NB:(half + 1) * NB],
                                    in_=pm[half * 4 + i][:])

                # v_s [s, (h dv)]: lhsT = vT chunks, rhs = wv chunks
                for spg in range(2):
                    pmv = [pp.tile([P, NB], f32, name=f"pmV{i}", tag="pp")
                           for i in range(8)]
                    for d in range(DC):
                        vc = w512.tile([P, NB], f32r, name="vcA", tag="w512")
                        nc.sync.dma_start(
                            vc[:], vT[e][d * P:(d + 1) * P, spg * NB:(spg + 1) * NB])
                        wvc = w1024.tile([P, H * DV], f32r, name="wvc", tag="w1024")
                        nc.sync.dma_start(wvc[:], wv[d * P:(d + 1) * P, :])
                        for s4 in range(4):
                            for n in range(2):
                                nc.tensor.matmul(
                                    pmv[s4 * 2 + n][:],
                                    vc[:, s4 * P:(s4 + 1) * P],
                                    wvc[:, n * NB:(n + 1) * NB],
                                    start=(d == 0), stop=(d == DC - 1))
                    for s4 in range(4):
                        for n in range(2):
                            nc.any.tensor_copy(
                                out=v_s[:, spg * 4 + s4, n * NB:(n + 1) * NB],
                                in_=pmv[s4 * 2 + n][:])

                # ---------------- Phase B+C: software-pipelined heads ----------------
                # PE stream per step i: scores(i+1) | sums(i) | attn@v(i-1)
                # so every PE instruction's inputs are ready when it
                # dispatches (no head-of-line blocking on exp/recip chains).
                iters = [(j, h) for j in range(2) for h in range(H)]
                NIT = len(iters)
                exp_tiles = {}
                concat_tiles = {}

                def stage_scores(i):
                    j, h = iters[i]
                    exp_t = big16.tile([P, SP, NB], f32r, name=f"exp_t_{e}_{i}",
                                       tag="big16")
                    for sp in range(SP):
                        psc = pp.tile([P, NB], f32, name=f"psc_{e}_{i}_{sp}", tag="pp")
                        for kc in range(2):
                            nc.tensor.matmul(
                                psc[:],
                                k_sT[:, h * 2 + kc, sp * P:(sp + 1) * P],
                                q_sT[:, h * 2 + kc, j * NB:(j + 1) * NB],
                                start=(kc == 0), stop=(kc == 1))
                        nc.scalar.activation(
                            out=exp_t[:, sp, :], in_=psc[:],
                            func=AF.Exp, scale=1.0 / 32.0)
                    exp_tiles[i] = exp_t

                def stage_sums(i):
                    j, h = iters[i]
                    exp_t = exp_tiles[i]
                    psum_row = pp.tile([1, NB], f32, name=f"prow_{e}_{i}", tag="pp")
                    for sp in range(SP):
                        nc.tensor.matmul(
                            psum_row[:], ones_t[:], exp_t[:, sp, :],
                            start=(sp == 0), stop=(sp == SP - 1))
                    srow = rows.tile([1, NB], f32, name=f"srow_{e}_{i}", tag="rows")
                    nc.scalar.copy(srow[:], psum_row[:])
                    rrow = rows.tile([1, NB], f32, name=f"rrow_{e}_{i}", tag="rows")
                    rscr = rows.tile([1, NB], f32, name=f"rscr_{e}_{i}", tag="rows")
                    nc.vector.reciprocal_approx_accurate(
                        out=rrow[:], in_=srow[:], scratch=rscr[:])
                    rbc = rbcp.tile([P, NB], f32, name=f"rbc_{e}_{i}", tag="rbc")
                    nc.gpsimd.partition_broadcast(rbc[:], rrow[:])
                    # normalize in place (attns_T); per-slice TTs so attn@v
                    # and the DMA can start before the whole tile is done
                    for sp in range(SP):
                        nc.vector.tensor_tensor(
                            exp_t[:, sp, :],
                            exp_t[:, sp, :].bitcast(f32),
                            rbc[:],
                            ALU.mult)
                    nc.sync.dma_start(
                        at_d[e, h].rearrange("(sp p) q -> p sp q", p=P)[
                            :, :, j * NB:(j + 1) * NB],
                        exp_t[:].bitcast(f32))

                def stage_attnv(i):
                    j, h = iters[i]
                    exp_t = exp_tiles.pop(i)
                    if j not in concat_tiles:
                        concat_tiles[j] = big16.tile(
                            [P, DC, NB], f32r, name=f"concat_{e}_{j}", tag="big16")
                    concat = concat_tiles[j]
                    for dt in range(2):
                        pov = pp.tile([P, NB], f32, name=f"pov_{e}_{i}_{dt}", tag="pp")
                        for sp in range(SP):
                            nc.tensor.matmul(
                                pov[:],
                                v_s[:, sp, h * DV + dt * P:h * DV + (dt + 1) * P],
                                exp_t[:, sp, :],
                                start=(sp == 0), stop=(sp == SP - 1))
                        nc.any.tensor_copy(
                            out=concat[:, h * 2 + dt, :], in_=pov[:])

                def phase_c(j):
                    concat = concat_tiles.pop(j)
                    pf = [pp.tile([P, NB], f32, name=f"pf_{e}_{j}_{i}", tag="pp")
                          for i in range(8)]
                    for c in range(DC):
                        pc = w1024.tile([P, D], f32r, name=f"pc_{e}_{j}_{c}",
                                        tag="w1024")
                        nc.sync.dma_start(pc[:], pw[c * P:(c + 1) * P, :])
                        for qt in range(QT4):
                            for n in range(2):
                                nc.tensor.matmul(
                                    pf[qt * 2 + n][:],
                                    concat[:, c, qt * P:(qt + 1) * P],
                                    pc[:, n * NB:(n + 1) * NB],
                                    start=(c == 0), stop=(c == DC - 1))
                    for qt in range(QT4):
                        row0 = j * NB + qt * P
                        qr = qrp.tile([P, D], f32, name=f"qr_{e}_{j}_{qt}", tag="qr")
                        nc.sync.dma_start(qr[:], qres[e][row0:row0 + P, :])
                        z = lnp.tile([P, D], f32, name=f"z_{e}_{j}_{qt}", tag="z")
                        for n in range(2):
                            nc.vector.tensor_tensor(
                                z[:, n * NB:(n + 1) * NB], pf[qt * 2 + n][:],
                                qr[:, n * NB:(n + 1) * NB], ALU.add)
                        if has_bias:
                            nc.vector.tensor_tensor(z[:], z[:], pb_bc[:], ALU.add)
                        st = statp.tile([P, 2, 6], f32, name=f"st_{e}_{j}_{qt}",
                                        tag="st")
                        for n in range(2):
                            nc.vector.bn_stats(
                                out=st[:, n, :], in_=z[:, n * NB:(n + 1) * NB])
                        mv = statp.tile([P, 2], f32, name=f"mv_{e}_{j}_{qt}", tag="mv")
                        nc.vector.bn_aggr(out=mv[:], in_=st[:])
                        # rstd = 1/(sqrt(var*N/(N-1)) + eps), with rsqrt via
                        # int bit-trick + 2 Newton steps (keeps ACT on the Exp
                        # table set only -- no table switching)
                        i32 = mybir.dt.int32
                        vk = statp.tile([P, 1], f32, name=f"vk_{e}_{j}_{qt}",
                                        tag="vk")
                        nc.vector.tensor_scalar_mul(vk[:], mv[:, 1:2],
                                                    float(D) / (D - 1))
                        yi = statp.tile([P, 1], f32, name=f"yi_{e}_{j}_{qt}",
                                        tag="yi")
                        nc.vector.tensor_scalar(
                            out=yi[:].bitcast(i32), in0=vk[:].bitcast(i32),
                            scalar1=1, scalar2=None,
                            op0=ALU.arith_shift_right)
                        nc.vector.tensor_scalar(
                            out=yi[:].bitcast(i32), in0=yi[:].bitcast(i32),
                            scalar1=-1, scalar2=0x5F3759DF,
                            op0=ALU.mult, op1=ALU.add)
                        t2 = statp.tile([P, 1], f32, name=f"t2_{e}_{j}_{qt}",
                                        tag="t2")
                        for _ in range(2):
                            nc.vector.tensor_tensor(t2[:], yi[:], yi[:], ALU.mult)
                            nc.vector.tensor_tensor(t2[:], t2[:], vk[:], ALU.mult)
                            nc.vector.tensor_scalar(
                                out=t2[:], in0=t2[:], scalar1=-0.5, scalar2=1.5,
                                op0=ALU.mult, op1=ALU.add)
                            nc.vector.tensor_tensor(yi[:], yi[:], t2[:], ALU.mult)
                        sg = statp.tile([P, 1], f32, name=f"sg_{e}_{j}_{qt}", tag="sg")
                        nc.vector.tensor_tensor(sg[:], vk[:], yi[:], ALU.mult)
                        nc.vector.tensor_scalar_add(sg[:], sg[:], LN_EPS)
                        rst = statp.tile([P, 1], f32, name=f"rst_{e}_{j}_{qt}",
                                         tag="rst")
                        nc.vector.reciprocal(rst[:], sg[:])
                        nc.vector.tensor_scalar(
                            out=z[:], in0=z[:], scalar1=mv[:, 0:1], scalar2=rst[:],
                            op0=ALU.subtract, op1=ALU.mult)
                        if has_ln:
                            nc.vector.tensor_tensor(z[:], z[:], la_bc[:], ALU.mult)
                            nc.vector.tensor_tensor(z[:], z[:], lb_bc[:], ALU.add)
                        nc.sync.dma_start(out_d[e][row0:row0 + P, :], z[:])

                stage_scores(0)
                for i in range(NIT):
                    if i + 1 < NIT:
                        stage_scores(i + 1)
                    stage_sums(i)
                    if i >= 1:
                        stage_attnv(i - 1)
                        if iters[i - 1][1] == H - 1:
                            phase_c(iters[i - 1][0])
                stage_attnv(NIT - 1)
                phase_c(iters[NIT - 1][0])

    nc.compile()
    return nc


def _get_nc(has_bias, has_ln):
    key = (has_bias, has_ln)
    if key not in _cache:
        _cache[key] = _build(has_bias, has_ln)
    return _cache[key]


def _run(in_maps, nc, trace=False, trace_kwargs=None):
    from concourse.bass_utils import run_bass_kernel_spmd

    return run_bass_kernel_spmd(
        nc, in_maps, list(range(NCORES)), trace=trace,
        trace_kwargs=trace_kwargs or {})


def _prep(q, k, v, w_qs, w_ks, w_vs, proj_w, proj_b, ln_a, ln_b):
    q = np.asarray(q, dtype=np.float32)
    k = np.asarray(k, dtype=np.float32)
    v = np.asarray(v, dtype=np.float32)
    w_qs = np.asarray(w_qs, dtype=np.float32)
    w_ks = np.asarray(w_ks, dtype=np.float32)
    w_vs = np.asarray(w_vs, dtype=np.float32)
    proj_w = np.ascontiguousarray(np.asarray(proj_w, dtype=np.float32))
    proj_b = np.asarray(proj_b, dtype=np.float32)
    ln_a = np.asarray(ln_a, dtype=np.float32)
    ln_b = np.asarray(ln_b, dtype=np.float32)

    has_bias = bool(np.any(proj_b != 0.0))
    has_ln = bool(np.any(ln_a != 1.0) or np.any(ln_b != 0.0))

    qT = np.ascontiguousarray(q.transpose(0, 2, 1))
    kT = np.ascontiguousarray(k.transpose(0, 2, 1))
    vT = np.ascontiguousarray(v.transpose(0, 2, 1))
    # [h, d, k] -> [d, (h k)]
    wq = np.ascontiguousarray(w_qs.transpose(1, 0, 2).reshape(D, H * DK))
    wk = np.ascontiguousarray(w_ks.transpose(1, 0, 2).reshape(D, H * DK))
    wv = np.ascontiguousarray(w_vs.transpose(1, 0, 2).reshape(D, H * DV))
    ones = np.ones((P, 1), np.float32)

    in_maps = []
    for c in range(NCORES):
        s = slice(c * BLOC, (c + 1) * BLOC)
        m = {"qT": qT[s], "kT": kT[s], "vT": vT[s], "qres": q[s],
             "wq": wq, "wk": wk, "wv": wv, "pw": proj_w, "ones": ones}
        if has_bias:
            m["pb"] = proj_b.reshape(1, D)
        if has_ln:
            m["la"] = ln_a.reshape(1, D)
            m["lb"] = ln_b.reshape(1, D)
        in_maps.append(m)
    return in_maps, has_bias, has_ln


def _gather(results):
    out = np.concatenate([r["out"] for r in results], axis=0)
    # attns_t per core: [BLOC, H, s, q] -> full [(h b), q, s]
    A = np.stack([r["attns_t"] for r in results], axis=0)  # [c, e, h, s, q]
    attns = np.ascontiguousarray(
        A.transpose(2, 0, 1, 4, 3).reshape(H * B, L, L))
    return out, attns


def kernel(**inputs):
    in_maps, has_bias, has_ln = _prep(**inputs)
    nc = _get_nc(has_bias, has_ln)
    res = _run(in_maps, nc)
    return _gather(res.results)
